# revision 31
# baseline (speedup 1.0000x reference)
"""Causal self-attention (B=4, T=2048, C=1024, H=16, Dh=64) on 8 trn2 NeuronCores.

Sharding: core = 2*b + g  (b = batch 0..3, g = head-group 0..1, 8 heads each).
Each core computes its batch's QKV projection for its 8 heads, causal
attention, and a partial out-projection; host sums the two head-group
partials per batch (the "all-reduce" of the tensor-parallel split).

v2 design (all matmul operands bf16 — 1 cyc/col vs fp32r's 2; PSUM stays f32):
  - J-outer schedule: k-projection prologue, then for each tq tile J the four
    head-pairs run S -> exp -> PV with FULL-ARRAY feeder matmuls (q-proj for
    the next head-pair, V-proj for the next J, out-proj for the previous J)
    interleaved every chunk.  Half-array attention matmuls (K=64 S, M=65 PV)
    alone leave the PE HAM activity monitor below its un-throttle threshold
    (measured: whole attention phase pinned at K=4/8 = 1.2 GHz); the
    interleave keeps genuine 128x128 work in every HAM window.
  - S head-pair tiles share one [128, 1024] PSUM tile (2 banks) so one ACT
    exp instruction covers both heads (ACT fixed cost ~350ns/inst dominates
    otherwise).  exp reads/writes strided 3D APs to skip the causal-masked
    left margin of diagonal tiles.
  - ACT runs exp ONLY; every PSUM->SBUF copy is on DVE; softmax denominator
    reciprocal broadcast via gpsimd partition_broadcast (no K=1 matmul).
  - PV with ones-augmented V (lhsT [tk,65]) -> y_aug^T[65, tq]; row 64
    accumulates the softmax denominator for free.
"""

import sys

for _p in ("/opt/trn_rl_repo", "/opt/pypackages"):
    if _p not in sys.path:
        sys.path.append(_p)

import numpy as np
from contextlib import ExitStack

import concourse.bass as bass
import concourse.tile as tile
from concourse import bacc, mybir
from concourse.bass_utils import run_bass_kernel_spmd

B, T, C = 4, 2048, 1024
H, DH = 16, 64
HG = 8          # heads per core
JW = 512        # tq tile width
NT = T // JW    # 4 tq tiles
NK = T // 128   # 16 tk tiles
NC_ = C // 128  # 8 c tiles
MASK_VAL = -1.0e5
F32 = mybir.dt.float32
BF16 = mybir.dt.bfloat16
EXP = mybir.ActivationFunctionType.Exp

_cache = {}


def _build(dbg=False):
    nc = bacc.Bacc("TRN2", target_bir_lowering=False, debug=False, num_devices=8)
    xT = nc.dram_tensor("xT", [C, T], BF16, kind="ExternalInput").ap()
    wqk = nc.dram_tensor("wqk", [C, 1024], BF16, kind="ExternalInput").ap()
    wv = nc.dram_tensor("wv", [C, 512], BF16, kind="ExternalInput").ap()
    wout = nc.dram_tensor("wout", [512, C], BF16, kind="ExternalInput").ap()
    dmask = nc.dram_tensor("dmask", [128, 128], F32, kind="ExternalInput").ap()
    ones_col = nc.dram_tensor("ones_col", [128, 1], BF16, kind="ExternalInput").ap()
    ones_row = nc.dram_tensor("ones_row", [1, 64], F32, kind="ExternalInput").ap()
    out = nc.dram_tensor("out", [T, C], BF16, kind="ExternalOutput").ap()
    if dbg:
        dq = nc.dram_tensor("dq", [512, T], BF16, kind="ExternalOutput").ap()
        dk = nc.dram_tensor("dk", [512, T], BF16, kind="ExternalOutput").ap()
        dv = nc.dram_tensor("dv", [NK * 128, HG * 65], BF16, kind="ExternalOutput").ap()
        dy = nc.dram_tensor("dy", [512, T], BF16, kind="ExternalOutput").ap()
        dden = nc.dram_tensor("dden", [2, JW], F32, kind="ExternalOutput").ap()
        dbc = nc.dram_tensor("dbc", [64, JW], F32, kind="ExternalOutput").ap()

    with tile.TileContext(nc) as tc:
        with ExitStack() as ctx:
            ctx.enter_context(nc.allow_low_precision(reason="bf16 matmuls intended"))

            # ---------------- persistent SBUF pools ----------------
            const_pool = ctx.enter_context(tc.tile_pool(name="const", bufs=1))
            x_pool = ctx.enter_context(tc.tile_pool(name="x", bufs=1))
            w_pool = ctx.enter_context(tc.tile_pool(name="w", bufs=1))
            qk_pool = ctx.enter_context(tc.tile_pool(name="qk", bufs=1))
            v_pool = ctx.enter_context(tc.tile_pool(name="v", bufs=1))
            y_pool = ctx.enter_context(tc.tile_pool(name="y", bufs=1))
            p_pool = ctx.enter_context(tc.tile_pool(name="p", bufs=6))
            rn_pool = ctx.enter_context(tc.tile_pool(name="rn", bufs=4))
            o_pool = ctx.enter_context(tc.tile_pool(name="o", bufs=4))
            # PSUM: s_pairs 2x2 banks + psy 2 + feeder 2 = 8 banks exactly
            s_psum = ctx.enter_context(tc.tile_pool(name="s_psum", bufs=2, space="PSUM"))
            y_psum = ctx.enter_context(tc.tile_pool(name="y_psum", bufs=2, space="PSUM"))
            f_psum = ctx.enter_context(tc.tile_pool(name="f_psum", bufs=2, space="PSUM"))

            dmask_sb = const_pool.tile([128, 128], F32, tag="dm", name="dmask_sb")
            onesb = const_pool.tile([128, 1], BF16, tag="ones", name="onesb")
            onesr = const_pool.tile([1, 64], mybir.dt.float32r, tag="onesr",
                                    name="onesr")
            nc.sync.dma_start(dmask_sb[:], dmask[:])
            nc.sync.dma_start(onesb[:], ones_col[:])
            nc.gpsimd.dma_start(onesr[:], ones_row[:])

            # x^T as 8x4 [128, 512] tiles (per c-tile, per tq-tile)
            xt = [[x_pool.tile([128, JW], BF16, tag=f"xt{ct}_{tt}", name=f"xt{ct}_{tt}")
                   for tt in range(NT)] for ct in range(NC_)]
            # weight tiles, all resident.  wk/wq batched per c-tile
            # ([128, 512] = all four m blocks in one DMA); matmuls slice them.
            wk_all = [w_pool.tile([128, 512], BF16, tag=f"wka{ct}", name=f"wka{ct}")
                      for ct in range(NC_)]
            wq_all = [w_pool.tile([128, 512], BF16, tag=f"wqa{ct}", name=f"wqa{ct}")
                      for ct in range(NC_)]
            wk = [[wk_all[ct][:, 128 * m:128 * m + 128] for ct in range(NC_)]
                  for m in range(4)]
            wq = [[wq_all[ct][:, 128 * m:128 * m + 128] for ct in range(NC_)]
                  for m in range(4)]
            wv_sb = [w_pool.tile([128, 512], BF16, tag=f"wv{ct}", name=f"wv{ct}") for ct in range(NC_)]
            wo_sb = [[w_pool.tile([128, 512], BF16, tag=f"wo{jt}_{et}", name=f"wo{jt}_{et}")
                      for et in range(2)] for jt in range(4)]

            # DMA: three queues (sync + scalar HWDGE, gpsimd SWDGE),
            # first-use order.  scalar only issues DMAs in the prologue,
            # before its exp stream begins.
            q3 = [nc.sync, nc.scalar, nc.gpsimd]
            for ct in range(NC_):    # k weights + x tt0, striped over 3 queues
                q3[ct % 3].dma_start(
                    wk_all[ct][:], wqk[128 * ct:128 * ct + 128, 512:1024])
                q3[(ct + 1) % 3].dma_start(
                    xt[ct][0][:], xT[128 * ct:128 * ct + 128, 0:JW])
            for ct in range(NC_):    # q weights on gpsimd, x tt1 on sync
                nc.gpsimd.dma_start(wq_all[ct][:],
                                    wqk[128 * ct:128 * ct + 128, 0:512])
                (nc.sync if ct % 2 == 0 else nc.scalar).dma_start(
                    xt[ct][1][:], xT[128 * ct:128 * ct + 128, JW:2 * JW])
            for ct in range(NC_):    # v weights + x tt2
                nc.sync.dma_start(wv_sb[ct][:], wv[128 * ct:128 * ct + 128, :])
                nc.scalar.dma_start(xt[ct][2][:],
                                    xT[128 * ct:128 * ct + 128, 2 * JW:3 * JW])
            for ct in range(NC_):    # x tt3
                nc.sync.dma_start(xt[ct][3][:],
                                  xT[128 * ct:128 * ct + 128, 3 * JW:4 * JW])
            for jt in range(4):
                for et in range(2):
                    nc.scalar.dma_start(
                        wo_sb[jt][et][:], wout[128 * jt:128 * jt + 128,
                                               512 * et:512 * et + 512])

            # persistent activation tensors
            k_sb = [qk_pool.tile([128, T], BF16, tag=f"k{m}", name=f"k_sb{m}")
                    for m in range(4)]
            q_sb = [qk_pool.tile([128, T], BF16, tag=f"q{m}", name=f"q_sb{m}")
                    for m in range(4)]
            v_sb = [v_pool.tile([128, HG * 65], BF16, tag=f"v{i}", name=f"v_sb{i}")
                    for i in range(NK)]
            y_sb = [y_pool.tile([128, T], BF16, tag=f"y{m}", name=f"y_sb{m}")
                    for m in range(4)]

            # ones column of each augmented V tile (written once)
            for i in range(NK):
                vv = v_sb[i][:].rearrange("p (h d) -> p h d", h=HG, d=65)
                a1, a2 = bass.broadcast_tensor_aps(
                    vv[:, :, 64:65],
                    onesb[:].rearrange("p (a b) -> p a b", a=1, b=1))
                nc.gpsimd.tensor_copy(a1, a2)

            # ---------------- emission helpers ----------------
            def qk_proj_mm(w_tiles, ct, tt, ps):
                nc.tensor.matmul(ps[:], w_tiles[ct][:], xt[ct][tt][:],
                                 start=(ct == 0), stop=(ct == NC_ - 1))

            def v_proj_mm(i, ct, ps):
                tt, r = divmod(i, 4)
                nc.tensor.matmul(ps[:], xt[ct][tt][:, 128 * r:128 * r + 128],
                                 wv_sb[ct][:],
                                 start=(ct == 0), stop=(ct == NC_ - 1))

            def o_proj_mm(it, et, jt, ps):
                nc.tensor.matmul(ps[:], y_sb[jt][:, 128 * it:128 * it + 128],
                                 wo_sb[jt][et][:],
                                 start=(jt == 0), stop=(jt == 3))

            # Feeder: a FIFO of "tiles", each a list of thunks that emit one
            # full-array matmul each (plus the PSUM copy-out on the last).
            # The PSUM tile is allocated by the first thunk, so f_psum bank
            # rotation follows emission order, and feed() only pops from the
            # front tile, so at most two feeder accumulations are in flight.
            pending = []  # list of (key, [thunks])

            def feed(n):
                while n > 0 and pending:
                    key, thunks = pending[0]
                    while n > 0 and thunks:
                        thunks.pop(0)()
                        n -= 1
                    if not thunks:
                        pending.pop(0)

            def require(key):
                while any(k == key for k, _ in pending):
                    _, thunks = pending[0]
                    while thunks:
                        thunks.pop(0)()
                    pending.pop(0)

            def push_q_proj(m, tt):
                cell = {}
                thunks = []
                for ct in range(NC_):
                    def mm(ct=ct, m=m, tt=tt, cell=cell):
                        if ct == 0:
                            cell["ps"] = f_psum.tile([128, JW], F32, tag="f",
                                                     name="fq")
                        ps = cell["ps"]
                        qk_proj_mm(wq[m], ct, tt, ps)
                        if ct == NC_ - 1:
                            nc.vector.tensor_copy(
                                q_sb[m][:, JW * tt:JW * tt + JW], ps[:])
                    thunks.append(mm)
                # q tiles are needed soonest: insert right after the front
                # tile (never split a partially-emitted tile).
                pending.insert(1 if pending else 0, (("q", m, tt), thunks))

            def push_v_proj(i):
                cell = {}
                thunks = []
                for ct in range(NC_):
                    def mm(ct=ct, i=i, cell=cell):
                        if ct == 0:
                            cell["ps"] = f_psum.tile([128, 512], F32, tag="f",
                                                     name="fv")
                        ps = cell["ps"]
                        v_proj_mm(i, ct, ps)
                        if ct == NC_ - 1:
                            nc.vector.tensor_copy(
                                v_sb[i][:].rearrange(
                                    "p (h d) -> p h d", h=HG, d=65)[:, :, 0:64],
                                ps[:].rearrange("p (h d) -> p h d", h=HG, d=64))
                    thunks.append(mm)
                pending.append((("v", i // 4), thunks))

            def push_o_proj(it, et):
                cell = {}
                thunks = []
                for jt in range(4):
                    def mm(jt=jt, it=it, et=et, cell=cell):
                        if jt == 0:
                            cell["ps"] = f_psum.tile([128, 512], F32, tag="f",
                                                     name="fo")
                        ps = cell["ps"]
                        o_proj_mm(it, et, jt, ps)
                        if jt == 3:
                            ot = o_pool.tile([128, 512], BF16, tag="ot", name="ot")
                            nc.vector.tensor_copy(ot[:], ps[:])
                            (nc.sync if (it + et) % 2 == 0 else nc.scalar).dma_start(
                                out[128 * it:128 * it + 128,
                                    512 * et:512 * et + 512], ot[:])
                    thunks.append(mm)
                pending.append((("o", it, et), thunks))

            # PE warm-up: dummy full-array f32 matmuls on the dmask const
            # during the DMA lead-in, parked on the psy ring (first real psy
            # allocation is ~40us in, so no WAR with the k-projection).
            warm_ps = y_psum.tile([128, 128], F32, tag="y", name="warm")
            for _ in range(14):
                nc.tensor.matmul(warm_ps[:], dmask_sb[:], dmask_sb[:],
                                 start=True, stop=True)

            # ---------------- prologue: k-projection (full-array, warms PE)
            for tt in range(NT):
                for m in range(4):
                    ps = f_psum.tile([128, JW], F32, tag="f", name="fk")
                    for ct in range(NC_):
                        qk_proj_mm(wk[m], ct, tt, ps)
                    nc.vector.tensor_copy(k_sb[m][:, JW * tt:JW * tt + JW], ps[:])
            # V tiles for J=0, and q tile (m=0, J=0), emitted directly
            for i in range(4):
                ps = f_psum.tile([128, 512], F32, tag="f", name="fv0")
                for ct in range(NC_):
                    v_proj_mm(i, ct, ps)
                nc.vector.tensor_copy(
                    v_sb[i][:].rearrange("p (h d) -> p h d", h=HG, d=65)[:, :, 0:64],
                    ps[:].rearrange("p (h d) -> p h d", h=HG, d=64))
            ps = f_psum.tile([128, JW], F32, tag="f", name="fq0")
            for ct in range(NC_):
                qk_proj_mm(wq[0], ct, 0, ps)
            nc.vector.tensor_copy(q_sb[0][:, 0:JW], ps[:])

            # ---------------- main loop: J outer, head-pair m inner -------
            out_ready = []
            for J in range(NT):
                nki = 4 * J + 4          # causal tk tiles for this J
                for m in range(4):
                    # stage feeder work (q first — needed soonest)
                    if m < 3:
                        push_q_proj(m + 1, J)
                    elif J < 3:
                        push_q_proj(0, J + 1)
                    if m == 0 and J < 3:
                        for i in range(4 * (J + 1), 4 * (J + 1) + 4):
                            push_v_proj(i)
                    # out-proj backlog: defer to J>=2 where feeder slots are
                    # plentiful (J=3 has no V/q work and starves otherwise)
                    n_out = 2 if J == 2 else (4 if J == 3 else 0)
                    for _ in range(min(n_out, len(out_ready))):
                        push_o_proj(*out_ready.pop(0))
                    # everything this (m, J) reads must be emitted by now
                    # (V tiles are required lazily, per PV chunk below)
                    if m > 0:
                        require(("q", m, J))
                    elif J > 0:
                        require(("q", 0, J))

                    if J == 0:
                        s_feed = pv_feed = 4
                    elif J == 3:
                        s_feed, pv_feed = ((3, 2) if m == 0 else
                                           ((2, 1) if m < 3 else (1, 1)))
                    else:
                        s_feed = pv_feed = 2
                    psy = {0: y_psum.tile([65, JW], F32, tag="y", name="psya"),
                           64: y_psum.tile([65, JW], F32, tag="y", name="psyb")}
                    CH = 2               # i-tiles per chunk
                    Ps = {}
                    done = []
                    for c0 in range(0, nki, CH):
                        chunk = list(range(c0, min(c0 + CH, nki)))
                        # S pair matmuls (half-array) ...
                        for i in chunk:
                            r = i - 4 * J
                            lo = 128 * r if r > 0 else 0
                            S2 = s_psum.tile([128, 2 * JW], F32, tag="s", name="S2")
                            for h, off in ((0, 0), (1, 64)):
                                nc.tensor.matmul(
                                    S2[:, h * JW + lo:h * JW + JW],
                                    k_sb[m][off:off + 64, 128 * i:128 * i + 128],
                                    q_sb[m][off:off + 64, JW * J + lo:JW * J + JW],
                                    start=True, stop=True)
                            if r >= 0:
                                sm = S2[:].rearrange(
                                    "p (h w) -> p h w", h=2,
                                    w=JW)[:, :, 128 * r:128 * r + 128]
                                ma, mb = bass.broadcast_tensor_aps(
                                    sm, dmask_sb[:].rearrange(
                                        "p (a w) -> p a w", a=1, w=128))
                                nc.vector.tensor_add(ma, ma, mb)
                            # one wide exp for both heads (strided 3D AP)
                            P2 = p_pool.tile([128, 2 * JW], BF16, tag="p", name="P2")
                            s3 = S2[:].rearrange("p (h w) -> p h w", h=2, w=JW)
                            p3 = P2[:].rearrange("p (h w) -> p h w", h=2, w=JW)
                            nc.scalar.activation(
                                p3[:, :, lo:JW], s3[:, :, lo:JW], EXP, scale=0.125)
                            Ps[i] = (P2, lo)
                        feed(s_feed)
                        # PV matmuls (K=128) for this chunk
                        for i in chunk:
                            require(("v", i // 4))
                        for i in chunk:
                            P2, lo = Ps.pop(i)
                            for h, off in ((0, 0), (1, 64)):
                                nc.tensor.matmul(
                                    psy[off][:, lo:JW],
                                    v_sb[i][:, 65 * (2 * m + h):65 * (2 * m + h) + 65],
                                    P2[:, h * JW + lo:h * JW + JW],
                                    start=(i == 0), stop=(i == nki - 1))
                        if c0 + CH < nki:
                            feed(pv_feed)

                    # tail: while the last normalize chain runs on DVE/Pool,
                    # keep the PE busy (and HAM warm) with jt=0..2 partial
                    # accumulations of four J=3 out-proj tiles.
                    tail_ps = {}
                    if J == 3 and m == 3:
                        for idx, (it, et) in enumerate(
                                [(12, 0), (12, 1), (13, 0), (13, 1)]):
                            pool, tg = ((f_psum, "f") if idx < 2 else
                                        (s_psum, "s"))
                            ps = pool.tile([128, 512], F32, tag=tg,
                                           name=f"tp{idx}")
                            tail_ps[(it, et)] = ps
                            for jt in range(3):
                                o_proj_mm(it, et, jt, ps)

                    # normalize: y = y_aug[0:64] * (1 / rowsum); both offs
                    # share one reciprocal + one partition_broadcast
                    den1 = rn_pool.tile([1, 2 * JW], F32, tag="den1", name="den1")
                    nc.vector.tensor_copy(den1[:, 0:JW], psy[0][64:65, :])
                    nc.vector.tensor_copy(den1[:, JW:2 * JW], psy[64][64:65, :])
                    rec = rn_pool.tile([1, 2 * JW], F32, tag="rec", name="rec")
                    nc.vector.reciprocal_approx_fast(rec[:], den1[:])
                    bcf = rn_pool.tile([64, 2 * JW], F32, tag="bcf", name="bcf")
                    nc.gpsimd.partition_broadcast(bcf[:], rec[:])
                    if dbg and m == 0 and J == 0:
                        nc.sync.dma_start(
                            dden[:], den1[:].rearrange("p (a w) -> (p a) w", a=2))
                        nc.sync.dma_start(dbc[:], bcf[:, 0:JW])
                    for h, off in ((0, 0), (1, 64)):
                        nc.vector.tensor_mul(
                            y_sb[m][off:off + 64, JW * J:JW * J + JW],
                            psy[off][0:64, :], bcf[:, h * JW:h * JW + JW])
                    feed(pv_feed)
                    for (it, et), ps in tail_ps.items():
                        o_proj_mm(it, et, 3, ps)
                        ot = o_pool.tile([128, 512], BF16, tag="ot", name="ot")
                        nc.vector.tensor_copy(ot[:], ps[:])
                        (nc.sync if (it + et) % 2 == 0 else nc.scalar).dma_start(
                            out[128 * it:128 * it + 128,
                                512 * et:512 * et + 512], ot[:])

                # after all m for this J: out-projection for J becomes ready
                for it in range(4 * J, 4 * J + 4):
                    for et in range(2):
                        if (it, et) not in ((12, 0), (12, 1), (13, 0), (13, 1)) \
                                or J != 3:
                            out_ready.append((it, et))
                if J == NT - 1:
                    for it, et in out_ready:
                        push_o_proj(it, et)
                    out_ready.clear()
                    while pending:       # drain tail directly
                        feed(8)
            assert not pending
            if dbg:
                for m in range(4):
                    nc.sync.dma_start(dq[128 * m:128 * m + 128, :], q_sb[m][:])
                    nc.sync.dma_start(dk[128 * m:128 * m + 128, :], k_sb[m][:])
                    nc.sync.dma_start(dy[128 * m:128 * m + 128, :], y_sb[m][:])
                for i in range(NK):
                    nc.sync.dma_start(dv[128 * i:128 * i + 128, :], v_sb[i][:])
    nc.compile()
    return nc


def _host_masks():
    a = np.arange(128, dtype=np.int64)[:, None]
    b = np.arange(128, dtype=np.int64)[None, :]
    return np.where(a <= b, np.float32(0.0), np.float32(MASK_VAL))


def _bf16(a):
    import ml_dtypes
    return np.ascontiguousarray(a.astype(ml_dtypes.bfloat16))


def _make_in_map(core, x, w_qkv, w_out):
    b, g = divmod(core, 2)
    import ml_dtypes
    xT = _bf16(np.ascontiguousarray(x[b].T))
    wqk = _bf16(np.concatenate(
        [w_qkv[:, 512 * g:512 * g + 512],
         w_qkv[:, 1024 + 512 * g:1024 + 512 * g + 512]], axis=1))
    wv = _bf16(w_qkv[:, 2048 + 512 * g:2048 + 512 * g + 512])
    wout_s = _bf16(w_out[512 * g:512 * g + 512, :])
    return dict(xT=xT, wqk=wqk, wv=wv, wout=wout_s,
                dmask=_host_masks(),
                ones_col=np.ones((128, 1), ml_dtypes.bfloat16),
                ones_row=np.ones((1, 64), np.float32))


def kernel(x, w_qkv, w_out):
    x = np.ascontiguousarray(x, dtype=np.float32)
    w_qkv = np.ascontiguousarray(w_qkv, dtype=np.float32)
    w_out = np.ascontiguousarray(w_out, dtype=np.float32)

    if "nc" not in _cache:
        _cache["nc"] = _build()
    nc = _cache["nc"]

    in_maps = [_make_in_map(core, x, w_qkv, w_out) for core in range(8)]

    res = run_bass_kernel_spmd(nc, in_maps, core_ids=list(range(8)))
    out = np.empty((B, T, C), np.float32)
    for b in range(B):
        out[b] = (res.results[2 * b]["out"].astype(np.float32)
                  + res.results[2 * b + 1]["out"].astype(np.float32))
    return out


# revision 32
# speedup vs baseline: 1.0099x; 1.0099x over previous
"""Causal self-attention (B=4, T=2048, C=1024, H=16, Dh=64) on 8 trn2 NeuronCores.

Sharding: core = 2*b + g  (b = batch 0..3, g = head-group 0..1, 8 heads each).
Each core computes its batch's QKV projection for its 8 heads, causal
attention, and a partial out-projection; host sums the two head-group
partials per batch (the "all-reduce" of the tensor-parallel split).

v2 design (all matmul operands bf16 — 1 cyc/col vs fp32r's 2; PSUM stays f32):
  - J-outer schedule: k-projection prologue, then for each tq tile J the four
    head-pairs run S -> exp -> PV with FULL-ARRAY feeder matmuls (q-proj for
    the next head-pair, V-proj for the next J, out-proj for the previous J)
    interleaved every chunk.  Half-array attention matmuls (K=64 S, M=65 PV)
    alone leave the PE HAM activity monitor below its un-throttle threshold
    (measured: whole attention phase pinned at K=4/8 = 1.2 GHz); the
    interleave keeps genuine 128x128 work in every HAM window.
  - S head-pair tiles share one [128, 1024] PSUM tile (2 banks) so one ACT
    exp instruction covers both heads (ACT fixed cost ~350ns/inst dominates
    otherwise).  exp reads/writes strided 3D APs to skip the causal-masked
    left margin of diagonal tiles.
  - ACT runs exp ONLY; every PSUM->SBUF copy is on DVE; softmax denominator
    reciprocal broadcast via gpsimd partition_broadcast (no K=1 matmul).
  - PV with ones-augmented V (lhsT [tk,65]) -> y_aug^T[65, tq]; row 64
    accumulates the softmax denominator for free.
"""

import sys

for _p in ("/opt/trn_rl_repo", "/opt/pypackages"):
    if _p not in sys.path:
        sys.path.append(_p)

import numpy as np
from contextlib import ExitStack

import concourse.bass as bass
import concourse.tile as tile
from concourse import bacc, mybir
from concourse.bass_utils import run_bass_kernel_spmd

B, T, C = 4, 2048, 1024
H, DH = 16, 64
HG = 8          # heads per core
JW = 512        # tq tile width
NT = T // JW    # 4 tq tiles
NK = T // 128   # 16 tk tiles
NC_ = C // 128  # 8 c tiles
MASK_VAL = -1.0e5
F32 = mybir.dt.float32
BF16 = mybir.dt.bfloat16
EXP = mybir.ActivationFunctionType.Exp

_cache = {}


def _build(dbg=False):
    nc = bacc.Bacc("TRN2", target_bir_lowering=False, debug=False, num_devices=8)
    xT = nc.dram_tensor("xT", [C, T], BF16, kind="ExternalInput").ap()
    wqk = nc.dram_tensor("wqk", [C, 1024], BF16, kind="ExternalInput").ap()
    wv = nc.dram_tensor("wv", [C, 512], BF16, kind="ExternalInput").ap()
    wout = nc.dram_tensor("wout", [512, C], BF16, kind="ExternalInput").ap()
    dmask = nc.dram_tensor("dmask", [128, 128], F32, kind="ExternalInput").ap()
    ones_col = nc.dram_tensor("ones_col", [128, 1], BF16, kind="ExternalInput").ap()
    ones_row = nc.dram_tensor("ones_row", [1, 64], F32, kind="ExternalInput").ap()
    out = nc.dram_tensor("out", [T, C], BF16, kind="ExternalOutput").ap()
    if dbg:
        dq = nc.dram_tensor("dq", [512, T], BF16, kind="ExternalOutput").ap()
        dk = nc.dram_tensor("dk", [512, T], BF16, kind="ExternalOutput").ap()
        dv = nc.dram_tensor("dv", [NK * 128, HG * 65], BF16, kind="ExternalOutput").ap()
        dy = nc.dram_tensor("dy", [512, T], BF16, kind="ExternalOutput").ap()
        dden = nc.dram_tensor("dden", [2, JW], F32, kind="ExternalOutput").ap()
        dbc = nc.dram_tensor("dbc", [64, JW], F32, kind="ExternalOutput").ap()

    with tile.TileContext(nc) as tc:
        with ExitStack() as ctx:
            ctx.enter_context(nc.allow_low_precision(reason="bf16 matmuls intended"))

            # ---------------- persistent SBUF pools ----------------
            const_pool = ctx.enter_context(tc.tile_pool(name="const", bufs=1))
            x_pool = ctx.enter_context(tc.tile_pool(name="x", bufs=1))
            w_pool = ctx.enter_context(tc.tile_pool(name="w", bufs=1))
            qk_pool = ctx.enter_context(tc.tile_pool(name="qk", bufs=1))
            v_pool = ctx.enter_context(tc.tile_pool(name="v", bufs=1))
            y_pool = ctx.enter_context(tc.tile_pool(name="y", bufs=1))
            p_pool = ctx.enter_context(tc.tile_pool(name="p", bufs=6))
            rn_pool = ctx.enter_context(tc.tile_pool(name="rn", bufs=4))
            o_pool = ctx.enter_context(tc.tile_pool(name="o", bufs=4))
            # PSUM: s_pairs 2x2 banks + psy 2 + feeder 2 = 8 banks exactly
            s_psum = ctx.enter_context(tc.tile_pool(name="s_psum", bufs=2, space="PSUM"))
            y_psum = ctx.enter_context(tc.tile_pool(name="y_psum", bufs=2, space="PSUM"))
            f_psum = ctx.enter_context(tc.tile_pool(name="f_psum", bufs=2, space="PSUM"))

            dmask_sb = const_pool.tile([128, 128], F32, tag="dm", name="dmask_sb")
            onesb = const_pool.tile([128, 1], BF16, tag="ones", name="onesb")
            onesr = const_pool.tile([1, 64], mybir.dt.float32r, tag="onesr",
                                    name="onesr")
            nc.sync.dma_start(dmask_sb[:], dmask[:])
            nc.sync.dma_start(onesb[:], ones_col[:])
            nc.gpsimd.dma_start(onesr[:], ones_row[:])

            # x^T as 8x4 [128, 512] tiles (per c-tile, per tq-tile)
            xt = [[x_pool.tile([128, JW], BF16, tag=f"xt{ct}_{tt}", name=f"xt{ct}_{tt}")
                   for tt in range(NT)] for ct in range(NC_)]
            # weight tiles, all resident.  wk/wq batched per c-tile
            # ([128, 512] = all four m blocks in one DMA); matmuls slice them.
            wk_all = [w_pool.tile([128, 512], BF16, tag=f"wka{ct}", name=f"wka{ct}")
                      for ct in range(NC_)]
            wq_all = [w_pool.tile([128, 512], BF16, tag=f"wqa{ct}", name=f"wqa{ct}")
                      for ct in range(NC_)]
            wk = [[wk_all[ct][:, 128 * m:128 * m + 128] for ct in range(NC_)]
                  for m in range(4)]
            wq = [[wq_all[ct][:, 128 * m:128 * m + 128] for ct in range(NC_)]
                  for m in range(4)]
            wv_sb = [w_pool.tile([128, 512], BF16, tag=f"wv{ct}", name=f"wv{ct}") for ct in range(NC_)]
            wo_sb = [[w_pool.tile([128, 512], BF16, tag=f"wo{jt}_{et}", name=f"wo{jt}_{et}")
                      for et in range(2)] for jt in range(4)]

            # DMA: three queues (sync + scalar HWDGE, gpsimd SWDGE),
            # first-use order.  scalar only issues DMAs in the prologue,
            # before its exp stream begins.
            q3 = [nc.sync, nc.scalar, nc.gpsimd]
            for ct in range(NC_):    # k weights + x tt0, striped over 3 queues
                q3[ct % 3].dma_start(
                    wk_all[ct][:], wqk[128 * ct:128 * ct + 128, 512:1024])
                q3[(ct + 1) % 3].dma_start(
                    xt[ct][0][:], xT[128 * ct:128 * ct + 128, 0:JW])
            for ct in range(NC_):    # q weights on gpsimd, x tt1 on sync
                nc.gpsimd.dma_start(wq_all[ct][:],
                                    wqk[128 * ct:128 * ct + 128, 0:512])
                (nc.sync if ct % 2 == 0 else nc.scalar).dma_start(
                    xt[ct][1][:], xT[128 * ct:128 * ct + 128, JW:2 * JW])
            for ct in range(NC_):    # v weights + x tt2
                nc.sync.dma_start(wv_sb[ct][:], wv[128 * ct:128 * ct + 128, :])
                nc.scalar.dma_start(xt[ct][2][:],
                                    xT[128 * ct:128 * ct + 128, 2 * JW:3 * JW])
            for ct in range(NC_):    # x tt3
                nc.sync.dma_start(xt[ct][3][:],
                                  xT[128 * ct:128 * ct + 128, 3 * JW:4 * JW])
            for jt in range(4):
                for et in range(2):
                    nc.scalar.dma_start(
                        wo_sb[jt][et][:], wout[128 * jt:128 * jt + 128,
                                               512 * et:512 * et + 512])

            # persistent activation tensors
            k_sb = [qk_pool.tile([128, T], BF16, tag=f"k{m}", name=f"k_sb{m}")
                    for m in range(4)]
            q_sb = [qk_pool.tile([128, T], BF16, tag=f"q{m}", name=f"q_sb{m}")
                    for m in range(4)]
            v_sb = [v_pool.tile([128, HG * 65], BF16, tag=f"v{i}", name=f"v_sb{i}")
                    for i in range(NK)]
            y_sb = [y_pool.tile([128, T], BF16, tag=f"y{m}", name=f"y_sb{m}")
                    for m in range(4)]

            # ones column of each augmented V tile (written once)
            for i in range(NK):
                vv = v_sb[i][:].rearrange("p (h d) -> p h d", h=HG, d=65)
                a1, a2 = bass.broadcast_tensor_aps(
                    vv[:, :, 64:65],
                    onesb[:].rearrange("p (a b) -> p a b", a=1, b=1))
                nc.gpsimd.tensor_copy(a1, a2)

            # ---------------- emission helpers ----------------
            def qk_proj_mm(w_tiles, ct, tt, ps):
                nc.tensor.matmul(ps[:], w_tiles[ct][:], xt[ct][tt][:],
                                 start=(ct == 0), stop=(ct == NC_ - 1))

            def v_proj_mm(i, ct, ps):
                tt, r = divmod(i, 4)
                nc.tensor.matmul(ps[:], xt[ct][tt][:, 128 * r:128 * r + 128],
                                 wv_sb[ct][:],
                                 start=(ct == 0), stop=(ct == NC_ - 1))

            def o_proj_mm(it, et, jt, ps):
                nc.tensor.matmul(ps[:], y_sb[jt][:, 128 * it:128 * it + 128],
                                 wo_sb[jt][et][:],
                                 start=(jt == 0), stop=(jt == 3))

            # Feeder: a FIFO of "tiles", each a list of thunks that emit one
            # full-array matmul each (plus the PSUM copy-out on the last).
            # The PSUM tile is allocated by the first thunk, so f_psum bank
            # rotation follows emission order, and feed() only pops from the
            # front tile, so at most two feeder accumulations are in flight.
            pending = []  # list of (key, [thunks])

            def feed(n):
                while n > 0 and pending:
                    key, thunks = pending[0]
                    while n > 0 and thunks:
                        thunks.pop(0)()
                        n -= 1
                    if not thunks:
                        pending.pop(0)

            def require(key):
                while any(k == key for k, _ in pending):
                    _, thunks = pending[0]
                    while thunks:
                        thunks.pop(0)()
                    pending.pop(0)

            def push_q_proj(m, tt):
                cell = {}
                thunks = []
                for ct in range(NC_):
                    def mm(ct=ct, m=m, tt=tt, cell=cell):
                        if ct == 0:
                            cell["ps"] = f_psum.tile([128, JW], F32, tag="f",
                                                     name="fq")
                        ps = cell["ps"]
                        qk_proj_mm(wq[m], ct, tt, ps)
                        if ct == NC_ - 1:
                            nc.vector.tensor_copy(
                                q_sb[m][:, JW * tt:JW * tt + JW], ps[:])
                    thunks.append(mm)
                # q tiles are needed soonest: insert right after the front
                # tile (never split a partially-emitted tile).
                pending.insert(1 if pending else 0, (("q", m, tt), thunks))

            def push_v_proj(i):
                cell = {}
                thunks = []
                for ct in range(NC_):
                    def mm(ct=ct, i=i, cell=cell):
                        if ct == 0:
                            cell["ps"] = f_psum.tile([128, 512], F32, tag="f",
                                                     name="fv")
                        ps = cell["ps"]
                        v_proj_mm(i, ct, ps)
                        if ct == NC_ - 1:
                            nc.vector.tensor_copy(
                                v_sb[i][:].rearrange(
                                    "p (h d) -> p h d", h=HG, d=65)[:, :, 0:64],
                                ps[:].rearrange("p (h d) -> p h d", h=HG, d=64))
                    thunks.append(mm)
                pending.append((("v", i // 4), thunks))

            def push_o_proj(it, et):
                cell = {}
                thunks = []
                for jt in range(4):
                    def mm(jt=jt, it=it, et=et, cell=cell):
                        if jt == 0:
                            cell["ps"] = f_psum.tile([128, 512], F32, tag="f",
                                                     name="fo")
                        ps = cell["ps"]
                        o_proj_mm(it, et, jt, ps)
                        if jt == 3:
                            ot = o_pool.tile([128, 512], BF16, tag="ot", name="ot")
                            nc.vector.tensor_copy(ot[:], ps[:])
                            (nc.sync if (it + et) % 2 == 0 else nc.scalar).dma_start(
                                out[128 * it:128 * it + 128,
                                    512 * et:512 * et + 512], ot[:])
                    thunks.append(mm)
                pending.append((("o", it, et), thunks))

            # ---------------- prologue: k-projection (full-array, warms PE)
            for tt in range(NT):
                for m in range(4):
                    ps = f_psum.tile([128, JW], F32, tag="f", name="fk")
                    for ct in range(NC_):
                        qk_proj_mm(wk[m], ct, tt, ps)
                    nc.vector.tensor_copy(k_sb[m][:, JW * tt:JW * tt + JW], ps[:])
            # V tiles for J=0, and q tile (m=0, J=0), emitted directly
            for i in range(4):
                ps = f_psum.tile([128, 512], F32, tag="f", name="fv0")
                for ct in range(NC_):
                    v_proj_mm(i, ct, ps)
                nc.vector.tensor_copy(
                    v_sb[i][:].rearrange("p (h d) -> p h d", h=HG, d=65)[:, :, 0:64],
                    ps[:].rearrange("p (h d) -> p h d", h=HG, d=64))
            ps = f_psum.tile([128, JW], F32, tag="f", name="fq0")
            for ct in range(NC_):
                qk_proj_mm(wq[0], ct, 0, ps)
            nc.vector.tensor_copy(q_sb[0][:, 0:JW], ps[:])

            # ---------------- main loop: J outer, head-pair m inner -------
            out_ready = []
            for J in range(NT):
                nki = 4 * J + 4          # causal tk tiles for this J
                for m in range(4):
                    # stage feeder work (q first — needed soonest)
                    if m < 3:
                        push_q_proj(m + 1, J)
                    elif J < 3:
                        push_q_proj(0, J + 1)
                    if m == 0 and J < 3:
                        for i in range(4 * (J + 1), 4 * (J + 1) + 4):
                            push_v_proj(i)
                    # out-proj backlog: defer to J>=2 where feeder slots are
                    # plentiful (J=3 has no V/q work and starves otherwise)
                    n_out = 2 if J == 2 else (4 if J == 3 else 0)
                    for _ in range(min(n_out, len(out_ready))):
                        push_o_proj(*out_ready.pop(0))
                    # everything this (m, J) reads must be emitted by now
                    # (V tiles are required lazily, per PV chunk below)
                    if m > 0:
                        require(("q", m, J))
                    elif J > 0:
                        require(("q", 0, J))

                    if J == 0:
                        s_feed = pv_feed = 4
                    elif J == 3:
                        s_feed, pv_feed = ((3, 2) if m == 0 else
                                           ((2, 1) if m < 3 else (1, 1)))
                    else:
                        s_feed = pv_feed = 2
                    psy = {0: y_psum.tile([65, JW], F32, tag="y", name="psya"),
                           64: y_psum.tile([65, JW], F32, tag="y", name="psyb")}
                    CH = 2               # i-tiles per chunk
                    Ps = {}
                    done = []
                    for c0 in range(0, nki, CH):
                        chunk = list(range(c0, min(c0 + CH, nki)))
                        # S pair matmuls (half-array) ...
                        for i in chunk:
                            r = i - 4 * J
                            lo = 128 * r if r > 0 else 0
                            S2 = s_psum.tile([128, 2 * JW], F32, tag="s", name="S2")
                            for h, off in ((0, 0), (1, 64)):
                                nc.tensor.matmul(
                                    S2[:, h * JW + lo:h * JW + JW],
                                    k_sb[m][off:off + 64, 128 * i:128 * i + 128],
                                    q_sb[m][off:off + 64, JW * J + lo:JW * J + JW],
                                    start=True, stop=True)
                            if r >= 0:
                                sm = S2[:].rearrange(
                                    "p (h w) -> p h w", h=2,
                                    w=JW)[:, :, 128 * r:128 * r + 128]
                                ma, mb = bass.broadcast_tensor_aps(
                                    sm, dmask_sb[:].rearrange(
                                        "p (a w) -> p a w", a=1, w=128))
                                nc.vector.tensor_add(ma, ma, mb)
                            # one wide exp for both heads (strided 3D AP)
                            P2 = p_pool.tile([128, 2 * JW], BF16, tag="p", name="P2")
                            s3 = S2[:].rearrange("p (h w) -> p h w", h=2, w=JW)
                            p3 = P2[:].rearrange("p (h w) -> p h w", h=2, w=JW)
                            nc.scalar.activation(
                                p3[:, :, lo:JW], s3[:, :, lo:JW], EXP, scale=0.125)
                            Ps[i] = (P2, lo)
                        feed(s_feed)
                        # PV matmuls (K=128) for this chunk
                        for i in chunk:
                            require(("v", i // 4))
                        for i in chunk:
                            P2, lo = Ps.pop(i)
                            for h, off in ((0, 0), (1, 64)):
                                nc.tensor.matmul(
                                    psy[off][:, lo:JW],
                                    v_sb[i][:, 65 * (2 * m + h):65 * (2 * m + h) + 65],
                                    P2[:, h * JW + lo:h * JW + JW],
                                    start=(i == 0), stop=(i == nki - 1))
                        if c0 + CH < nki:
                            feed(pv_feed)

                    # tail: while the last normalize chain runs on DVE/Pool,
                    # keep the PE busy (and HAM warm) with jt=0..2 partial
                    # accumulations of four J=3 out-proj tiles.
                    tail_ps = {}
                    if J == 3 and m == 3:
                        for idx, (it, et) in enumerate(
                                [(12, 0), (12, 1), (13, 0), (13, 1)]):
                            pool, tg = ((f_psum, "f") if idx < 2 else
                                        (s_psum, "s"))
                            ps = pool.tile([128, 512], F32, tag=tg,
                                           name=f"tp{idx}")
                            tail_ps[(it, et)] = ps
                            for jt in range(3):
                                o_proj_mm(it, et, jt, ps)

                    # normalize: y = y_aug[0:64] * (1 / rowsum); both offs
                    # share one reciprocal + one partition_broadcast
                    den1 = rn_pool.tile([1, 2 * JW], F32, tag="den1", name="den1")
                    nc.vector.tensor_copy(den1[:, 0:JW], psy[0][64:65, :])
                    nc.vector.tensor_copy(den1[:, JW:2 * JW], psy[64][64:65, :])
                    rec = rn_pool.tile([1, 2 * JW], F32, tag="rec", name="rec")
                    nc.vector.reciprocal_approx_fast(rec[:], den1[:])
                    bcf = rn_pool.tile([64, 2 * JW], F32, tag="bcf", name="bcf")
                    nc.gpsimd.partition_broadcast(bcf[:], rec[:])
                    if dbg and m == 0 and J == 0:
                        nc.sync.dma_start(
                            dden[:], den1[:].rearrange("p (a w) -> (p a) w", a=2))
                        nc.sync.dma_start(dbc[:], bcf[:, 0:JW])
                    for h, off in ((0, 0), (1, 64)):
                        nc.vector.tensor_mul(
                            y_sb[m][off:off + 64, JW * J:JW * J + JW],
                            psy[off][0:64, :], bcf[:, h * JW:h * JW + JW])
                    feed(pv_feed)
                    for (it, et), ps in tail_ps.items():
                        o_proj_mm(it, et, 3, ps)
                        ot = o_pool.tile([128, 512], BF16, tag="ot", name="ot")
                        nc.vector.tensor_copy(ot[:], ps[:])
                        (nc.sync if (it + et) % 2 == 0 else nc.scalar).dma_start(
                            out[128 * it:128 * it + 128,
                                512 * et:512 * et + 512], ot[:])

                # after all m for this J: out-projection for J becomes ready
                for it in range(4 * J, 4 * J + 4):
                    for et in range(2):
                        if (it, et) not in ((12, 0), (12, 1), (13, 0), (13, 1)) \
                                or J != 3:
                            out_ready.append((it, et))
                if J == NT - 1:
                    for it, et in out_ready:
                        push_o_proj(it, et)
                    out_ready.clear()
                    while pending:       # drain tail directly
                        feed(8)
            assert not pending
            if dbg:
                for m in range(4):
                    nc.sync.dma_start(dq[128 * m:128 * m + 128, :], q_sb[m][:])
                    nc.sync.dma_start(dk[128 * m:128 * m + 128, :], k_sb[m][:])
                    nc.sync.dma_start(dy[128 * m:128 * m + 128, :], y_sb[m][:])
                for i in range(NK):
                    nc.sync.dma_start(dv[128 * i:128 * i + 128, :], v_sb[i][:])
    nc.compile()
    return nc


def _host_masks():
    a = np.arange(128, dtype=np.int64)[:, None]
    b = np.arange(128, dtype=np.int64)[None, :]
    return np.where(a <= b, np.float32(0.0), np.float32(MASK_VAL))


def _bf16(a):
    import ml_dtypes
    return np.ascontiguousarray(a.astype(ml_dtypes.bfloat16))


def _make_in_map(core, x, w_qkv, w_out):
    b, g = divmod(core, 2)
    import ml_dtypes
    xT = _bf16(np.ascontiguousarray(x[b].T))
    wqk = _bf16(np.concatenate(
        [w_qkv[:, 512 * g:512 * g + 512],
         w_qkv[:, 1024 + 512 * g:1024 + 512 * g + 512]], axis=1))
    wv = _bf16(w_qkv[:, 2048 + 512 * g:2048 + 512 * g + 512])
    wout_s = _bf16(w_out[512 * g:512 * g + 512, :])
    return dict(xT=xT, wqk=wqk, wv=wv, wout=wout_s,
                dmask=_host_masks(),
                ones_col=np.ones((128, 1), ml_dtypes.bfloat16),
                ones_row=np.ones((1, 64), np.float32))


def kernel(x, w_qkv, w_out):
    x = np.ascontiguousarray(x, dtype=np.float32)
    w_qkv = np.ascontiguousarray(w_qkv, dtype=np.float32)
    w_out = np.ascontiguousarray(w_out, dtype=np.float32)

    if "nc" not in _cache:
        _cache["nc"] = _build()
    nc = _cache["nc"]

    in_maps = [_make_in_map(core, x, w_qkv, w_out) for core in range(8)]

    res = run_bass_kernel_spmd(nc, in_maps, core_ids=list(range(8)))
    out = np.empty((B, T, C), np.float32)
    for b in range(B):
        out[b] = (res.results[2 * b]["out"].astype(np.float32)
                  + res.results[2 * b + 1]["out"].astype(np.float32))
    return out
